# revision 8
# baseline (speedup 1.0000x reference)
"""DeepSeek-style MLA attention block on 8 Trainium2 NeuronCores.

Sharding: tensor-parallel over the 16 heads (2 heads per core) through
attention; AllToAll re-shards to token-parallel (512 tokens per core) for
the output projection; host concatenates the 8 token blocks.

Math notes (exact, no approximation):
  - g_q / g_kv layernorm gains are folded into the rows of w_qa / w_kva on
    the host, so the device computes a single shared normalized x-hat.
  - 1/sqrt(head_dim) is folded into the Exp activation's scale.
  - Softmax skips the max-subtraction: logits are O(0.1) for this module's
    weight scale (LN output @ 0.02-scale low-rank chain), so exp() is far
    from overflow; the kernel asserts the PSUM logits implicitly through
    the final rel-err check in testing.
  - Per-(head,token) softmax normalization is applied to y^T before the
    AllToAll via a ones-matmul partition-broadcast of 1/sum.

Device layout: activations live feature-major ("x^T": feature on the SBUF
partition axis, token on the free axis) so every projection feeds the PE
array directly; x-hat is transposed once per 128x128 block on the PE.
Matmuls run in float32r (full-rate fp32 PE mode).
"""

import numpy as np

E = 2048
H = 16
D = 128
QR = 64
KR = 128
RD = 64
EPS = 1e-5
ROPE_BASE = 10000.0
B = 2
S = 2048
T = B * S            # 4096 total token rows
NCORES = 8
HPC = H // NCORES    # heads per core = 2
TBLK = T // NCORES   # tokens per core after re-shard = 512
NCH = 8              # token chunks of 512 in phase 1
CHK = 512
EXP_SCALE = float(1.0 / np.sqrt(D))

_CACHE = {}


def _build_module():
    import concourse.mybir as mybir
    import concourse.tile as tile
    from concourse import bacc
    from concourse.masks import make_identity

    F32 = mybir.dt.float32
    F32R = mybir.dt.float32r
    AF = mybir.ActivationFunctionType
    ALU = mybir.AluOpType

    nc = bacc.Bacc("TRN2", target_bir_lowering=False, debug=False,
                   num_devices=NCORES)

    x_d = nc.declare_dram_parameter("x", [T, E], F32, isOutput=False)
    wqa_d = nc.declare_dram_parameter("wqa", [E, QR], F32R, isOutput=False)
    wqb_d = nc.declare_dram_parameter("wqb", [QR, HPC * D], F32R, isOutput=False)
    wkc_d = nc.declare_dram_parameter("wkc", [E, KR], F32R, isOutput=False)
    wv_d = nc.declare_dram_parameter("wv", [E, HPC * D], F32R, isOutput=False)
    wkvb_d = nc.declare_dram_parameter("wkvb", [KR, HPC * D], F32R, isOutput=False)
    wo_d = nc.declare_dram_parameter("wo", [E, E], F32R, isOutput=False)
    cos_d = nc.declare_dram_parameter("cos_t", [RD // 2, T], F32, isOutput=False)
    sin_d = nc.declare_dram_parameter("sin_t", [RD // 2, T], F32, isOutput=False)
    mask_d = nc.declare_dram_parameter("masks", [128, 4, CHK], F32, isOutput=False)
    out_d = nc.declare_dram_parameter("out", [TBLK, E], F32, isOutput=True)

    with tile.TileContext(nc) as tc:
        with (
            tc.tile_pool(name="cons", bufs=1) as cons,
            tc.tile_pool(name="qkv", bufs=1) as qkv,
            tc.tile_pool(name="dram", bufs=1, space="DRAM") as dram,
        ):
            ident = cons.tile([128, 128], F32)
            make_identity(nc, ident)
            ones32a = cons.tile([128, 1], F32)
            nc.vector.memset(ones32a, 1.0)
            ones_col = cons.tile([128, 1], F32R)
            nc.vector.tensor_copy(out=ones_col, in_=ones32a)
            ones32b = cons.tile([1, 128], F32)
            nc.vector.memset(ones32b, 1.0)
            ones_row = cons.tile([1, 128], F32R)
            nc.vector.tensor_copy(out=ones_row, in_=ones32b)
            eps_t = cons.tile([128, 1], F32)
            nc.vector.memset(eps_t, EPS)
            masks = cons.tile([128, 4, CHK], F32)
            nc.sync.dma_start(out=masks, in_=mask_d.ap())
            cos_h = cons.tile([RD // 2, T], F32)
            nc.sync.dma_start(out=cos_h, in_=cos_d.ap())
            sin_h = cons.tile([RD // 2, T], F32)
            nc.sync.dma_start(out=sin_h, in_=sin_d.ap())

            wqa = cons.tile([128, E // 128, QR], F32R)
            nc.sync.dma_start(out=wqa, in_=wqa_d.ap().rearrange("(k p) m -> p k m", p=128))
            wkc = cons.tile([128, E // 128, KR], F32R)
            nc.sync.dma_start(out=wkc, in_=wkc_d.ap().rearrange("(k p) m -> p k m", p=128))
            wv = cons.tile([128, E // 128, HPC * D], F32R)
            nc.sync.dma_start(out=wv, in_=wv_d.ap().rearrange("(k p) m -> p k m", p=128))
            wqb = cons.tile([QR, HPC * D], F32R)
            nc.sync.dma_start(out=wqb, in_=wqb_d.ap())
            wkvb = cons.tile([KR, HPC * D], F32R)
            nc.sync.dma_start(out=wkvb, in_=wkvb_d.ap())

            # head-resident projections: feature-major Q^T, K^T; token-major V
            qT = [qkv.tile([128, T], F32R, name=f"qT{h}") for h in range(HPC)]
            kT = [qkv.tile([128, T], F32R, name=f"kT{h}") for h in range(HPC)]

            vtm_d = dram.tile([T // 128, 128, HPC * D], F32R)
            a2a_in = dram.tile([NCORES, HPC * D, TBLK], F32R)
            a2a_out = dram.tile([NCORES, HPC * D, TBLK], F32R)

            # ---------------- phase 1: LN -> x-hat^T -> projections ----------
            with (
                tc.tile_pool(name="p1sb", bufs=2) as p1,
                tc.tile_pool(name="p1ps", bufs=1, space="PSUM") as ps1,
            ):
                for tcix in range(NCH):
                    xhT = p1.tile([128, E // 128, CHK], F32R, tag="xhT", bufs=1)
                    for xt in range(4):
                        row0 = tcix * CHK + xt * 128
                        x_t = p1.tile([128, E], F32, tag="x")
                        nc.sync.dma_start(out=x_t, in_=x_d.ap()[row0:row0 + 128, :])
                        stats = p1.tile([128, 4, 6], F32, tag="stats")
                        for g in range(4):
                            nc.vector.bn_stats(out=stats[:, g, :],
                                               in_=x_t[:, g * 512:(g + 1) * 512])
                        mv = p1.tile([128, 2], F32, tag="mv")
                        nc.vector.bn_aggr(out=mv, in_=stats)
                        std = p1.tile([128, 1], F32, tag="std")
                        nc.scalar.activation(out=std, in_=mv[:, 1:2], func=AF.Sqrt,
                                             bias=eps_t[:])
                        rstd = p1.tile([128, 1], F32, tag="rstd")
                        nc.vector.reciprocal(out=rstd, in_=std)
                        nmu = p1.tile([128, 1], F32, tag="nmu")
                        nc.vector.tensor_tensor(out=nmu, in0=mv[:, 0:1], in1=rstd,
                                                op=ALU.mult)
                        nc.vector.tensor_scalar(out=nmu, in0=nmu, scalar1=-1.0,
                                                scalar2=None, op0=ALU.mult)
                        xhat = x_t
                        nc.vector.tensor_scalar(out=xhat, in0=x_t, scalar1=rstd[:],
                                                scalar2=nmu[:], op0=ALU.mult,
                                                op1=ALU.add)
                        for j in range(E // 128):
                            tp = ps1.tile([128, 128], F32, tag="tp", bufs=2)
                            nc.tensor.transpose(tp[:], xhat[:, j * 128:(j + 1) * 128],
                                                ident[:])
                            if j % 2 == 0:
                                nc.vector.tensor_copy(
                                    out=xhT[:, j, xt * 128:(xt + 1) * 128], in_=tp)
                            else:
                                nc.scalar.copy(
                                    out=xhT[:, j, xt * 128:(xt + 1) * 128], in_=tp)

                    # stage 1 projections for this 512-token chunk
                    mm_ps = {
                        "qlow": ps1.tile([QR, CHK], F32, tag="mm0", bufs=1, name="ps_qlow"),
                        "kc": ps1.tile([128, CHK], F32, tag="mm1", bufs=1, name="ps_kc"),
                        "v0": ps1.tile([128, CHK], F32, tag="mm2", bufs=1, name="ps_v0"),
                        "v1": ps1.tile([128, CHK], F32, tag="mm3", bufs=1, name="ps_v1"),
                    }
                    for k in range(E // 128):
                        rhs = xhT[:, k, :]
                        st = (k == 0)
                        sp = (k == E // 128 - 1)
                        nc.tensor.matmul(mm_ps["qlow"][:], wqa[:, k, :],
                                         rhs, start=st, stop=sp)
                        nc.tensor.matmul(mm_ps["kc"][:], wkc[:, k, :],
                                         rhs, start=st, stop=sp)
                        nc.tensor.matmul(mm_ps["v0"][:], wv[:, k, 0:128],
                                         rhs, start=st, stop=sp)
                        nc.tensor.matmul(mm_ps["v1"][:], wv[:, k, 128:256],
                                         rhs, start=st, stop=sp)

                    qlow_sb = p1.tile([QR, CHK], F32R, tag="qlow_sb", bufs=1)
                    nc.scalar.copy(out=qlow_sb, in_=mm_ps["qlow"])
                    kc_sb = p1.tile([128, CHK], F32R, tag="kc_sb", bufs=1)
                    nc.scalar.copy(out=kc_sb, in_=mm_ps["kc"])

                    # V: transpose feature-major v^T blocks to token-major, spill to DRAM
                    for h in range(2):
                        v_sb = p1.tile([128, CHK], F32, tag=f"v_sb{h}", bufs=1)
                        nc.scalar.copy(out=v_sb, in_=mm_ps[f"v{h}"])
                        for i in range(4):
                            tpv = ps1.tile([128, 128], F32, tag="tp", bufs=2)
                            nc.tensor.transpose(tpv[:], v_sb[:, i * 128:(i + 1) * 128],
                                                ident[:])
                            v_out = p1.tile([128, 128], F32R, tag="v_out", bufs=3)
                            nc.vector.tensor_copy(out=v_out, in_=tpv)
                            nc.sync.dma_start(
                                out=vtm_d[tcix * 4 + i, :, h * 128:(h + 1) * 128],
                                in_=v_out)

                    # stage 2: q = wqb^T @ qlow, k = wkvb^T @ kc; then RoPE
                    for h in range(2):
                        q_ps = ps1.tile([128, CHK], F32, tag=f"mm{2 * h}", bufs=1)
                        nc.tensor.matmul(q_ps[:],
                                         wqb[:, h * 128:(h + 1) * 128],
                                         qlow_sb[:],
                                         start=True, stop=True)
                        k_ps = ps1.tile([128, CHK], F32, tag=f"mm{2 * h + 1}", bufs=1)
                        nc.tensor.matmul(k_ps[:],
                                         wkvb[:, h * 128:(h + 1) * 128],
                                         kc_sb[:],
                                         start=True, stop=True)
                        cs = slice(tcix * CHK, (tcix + 1) * CHK)
                        for src_ps, dstT in ((q_ps, qT[h]), (k_ps, kT[h])):
                            m1 = p1.tile([RD, CHK], F32, tag="rope_m1", bufs=1)
                            nc.vector.tensor_tensor(out=m1[0:32, :], in0=src_ps[0:32, :],
                                                    in1=cos_h[:, cs], op=ALU.mult)
                            nc.vector.tensor_tensor(out=m1[32:64, :], in0=src_ps[32:64, :],
                                                    in1=cos_h[:, cs], op=ALU.mult)
                            t2 = p1.tile([RD, CHK], F32, tag="rope_t2", bufs=1)
                            nc.vector.tensor_tensor(out=t2[0:32, :],
                                                    in0=src_ps[32:64, :],
                                                    in1=sin_h[:, cs], op=ALU.mult)
                            nc.vector.tensor_tensor(out=t2[32:64, :],
                                                    in0=src_ps[0:32, :],
                                                    in1=sin_h[:, cs], op=ALU.mult)
                            nc.vector.tensor_tensor(out=dstT[0:32, cs], in0=m1[0:32, :],
                                                    in1=t2[0:32, :], op=ALU.subtract)
                            nc.vector.tensor_tensor(out=dstT[32:64, cs], in0=m1[32:64, :],
                                                    in1=t2[32:64, :], op=ALU.add)
                            nc.scalar.copy(out=dstT[RD:128, cs], in_=src_ps[RD:128, :])

            # ---------------- phase 2: causal attention per (batch, head) ----
            with (
                tc.tile_pool(name="p2sb", bufs=1) as p2,
                tc.tile_pool(name="p2ps", bufs=1, space="PSUM") as ps2,
            ):
                for b in range(B):
                    for h in range(HPC):
                        boff = b * S
                        # process query chunks in pairs to stay within 8 PSUM
                        # banks: s(2) + y(2) + sums(2) + bc(shares s)
                        for g in range(2):
                            grp = [2 * g, 2 * g + 1]
                            y_ps = {qc: ps2.tile([128, CHK], F32, tag=f"y{qc % 2}",
                                                 bufs=1, name=f"ps_y{qc}")
                                    for qc in grp}
                            sums_ps = {qc: ps2.tile([1, CHK], F32, tag=f"sums{qc % 2}",
                                                    bufs=1, name=f"ps_sums{qc}")
                                       for qc in grp}
                            kt_max = 4 * grp[-1] + 3
                            for kt in range(kt_max + 1):
                                kslc = slice(boff + kt * 128, boff + (kt + 1) * 128)
                                qcs = [qc for qc in grp if kt <= 4 * qc + 3]
                                att = {}
                                for qc in qcs:
                                    qslc = slice(boff + qc * CHK, boff + (qc + 1) * CHK)
                                    s_ps = ps2.tile([128, CHK], F32, tag="s", bufs=2)
                                    nc.tensor.matmul(s_ps[:],
                                                     kT[h][:, kslc],
                                                     qT[h][:, qslc],
                                                     start=True, stop=True)
                                    a_t = p2.tile([128, CHK], F32R, tag=f"att{qc % 2}",
                                                  bufs=2, name=f"att{qc}")
                                    nc.scalar.activation(out=a_t, in_=s_ps,
                                                         func=AF.Exp,
                                                         scale=EXP_SCALE)
                                    d = kt - 4 * qc
                                    if 0 <= d <= 3:
                                        nc.vector.tensor_tensor(out=a_t, in0=a_t,
                                                                in1=masks[:, d, :],
                                                                op=ALU.mult)
                                    att[qc] = a_t
                                for qc in qcs:
                                    nc.tensor.matmul(
                                        sums_ps[qc][:], ones_col[:], att[qc][:],
                                        start=(kt == 0), stop=(kt == 4 * qc + 3))
                                ktile = b * 16 + kt
                                v_t = p2.tile([128, 128], F32R, tag="v_t", bufs=3)
                                nc.sync.dma_start(
                                    out=v_t,
                                    in_=vtm_d[ktile, :, h * 128:(h + 1) * 128])
                                for qc in qcs:
                                    nc.tensor.matmul(
                                        y_ps[qc][:],
                                        v_t[:],
                                        att[qc][:],
                                        start=(kt == 0), stop=(kt == 4 * qc + 3))

                            for qc in grp:
                                recip = p2.tile([1, CHK], F32, tag="recip", bufs=2)
                                nc.vector.reciprocal(out=recip, in_=sums_ps[qc])
                                recip_r = p2.tile([1, CHK], F32R, tag="recipr", bufs=2)
                                nc.scalar.copy(out=recip_r, in_=recip)
                                bc_ps = ps2.tile([128, CHK], F32, tag="s", bufs=2)
                                nc.tensor.matmul(bc_ps[:], ones_row[:],
                                                 recip_r[:],
                                                 start=True, stop=True)
                                bc_sb = p2.tile([128, CHK], F32, tag="bc", bufs=2)
                                nc.scalar.copy(out=bc_sb, in_=bc_ps)
                                ynorm = p2.tile([128, CHK], F32R, tag="ynorm", bufs=2)
                                nc.vector.tensor_tensor(out=ynorm, in0=y_ps[qc],
                                                        in1=bc_sb, op=ALU.mult)
                                jblk = b * 4 + qc
                                nc.sync.dma_start(
                                    out=a2a_in[jblk, h * 128:(h + 1) * 128, :],
                                    in_=ynorm)

            # ---------------- phase 3: AllToAll re-shard --------------------
            import os as _os
            if _os.environ.get("KERNEL_NO_COLLECTIVE"):
                # cost-model variant: local copy stands in for the A2A
                nc.sync.dma_start(out=a2a_out[:], in_=a2a_in[:])
            else:
                nc.gpsimd.collective_compute(
                    "AllToAll", mybir.AluOpType.bypass,
                    replica_groups=[list(range(NCORES))],
                    ins=[a2a_in.opt()],
                    outs=[a2a_out.opt()],
                )

            # ---------------- phase 4: output projection --------------------
            with (
                tc.tile_pool(name="p4sb", bufs=1) as p4,
                tc.tile_pool(name="p4w", bufs=3) as p4w,
                tc.tile_pool(name="p4ps", bufs=1, space="PSUM") as ps4,
            ):
                ya = p4.tile([128, E // 128, TBLK], F32R)
                nc.sync.dma_start(
                    out=ya, in_=a2a_out[:].rearrange("c (a p) t -> p (c a) t", p=128))
                for half in range(2):
                    o_ps = [[ps4.tile([128, 512], F32, tag=f"o{mt}{nt}", bufs=1, name=f"ps_o{mt}{nt}")
                             for nt in range(2)] for mt in range(4)]
                    for kt in range(16):
                        wo_t = p4w.tile([128, 1024], F32R, tag="wo")
                        nc.sync.dma_start(
                            out=wo_t,
                            in_=wo_d.ap()[kt * 128:(kt + 1) * 128,
                                          half * 1024:(half + 1) * 1024])
                        for mt in range(4):
                            for nt in range(2):
                                nc.tensor.matmul(
                                    o_ps[mt][nt][:],
                                    ya[:, kt, mt * 128:(mt + 1) * 128],
                                    wo_t[:, nt * 512:(nt + 1) * 512],
                                    start=(kt == 0), stop=(kt == 15))
                    for mt in range(4):
                        for nt in range(2):
                            o_sb = p4.tile([128, 512], F32, tag="o_sb", bufs=4)
                            if (mt + nt) % 2 == 0:
                                nc.scalar.copy(out=o_sb, in_=o_ps[mt][nt])
                            else:
                                nc.vector.tensor_copy(out=o_sb, in_=o_ps[mt][nt])
                            nc.sync.dma_start(
                                out=out_d.ap()[mt * 128:(mt + 1) * 128,
                                               half * 1024 + nt * 512:
                                               half * 1024 + (nt + 1) * 512],
                                in_=o_sb)

    nc.compile()
    return nc


def _host_inputs(x, g_q, g_kv, w_qa, w_qb, w_kva, w_kvb, w_o):
    x_flat = np.ascontiguousarray(x.reshape(T, E), dtype=np.float32)
    wqa_g = np.ascontiguousarray(w_qa * g_q[:, None], dtype=np.float32)
    wkva_g = w_kva * g_kv[:, None]
    wkc = np.ascontiguousarray(wkva_g[:, :KR], dtype=np.float32)
    wo = np.ascontiguousarray(w_o, dtype=np.float32)

    inv_freq = 1.0 / (ROPE_BASE ** (np.arange(0, RD, 2, dtype=np.float32) / RD))
    freqs = np.arange(S, dtype=np.float32)[:, None] * inv_freq[None, :]  # [S, 32]
    cos_t = np.ascontiguousarray(np.tile(np.cos(freqs).T, (1, B)), dtype=np.float32)
    sin_t = np.ascontiguousarray(np.tile(np.sin(freqs).T, (1, B)), dtype=np.float32)

    ii = np.arange(128)[:, None, None]
    dd = np.arange(4)[None, :, None]
    jj = np.arange(CHK)[None, None, :]
    masks = ((ii + 128 * dd) <= jj).astype(np.float32)

    in_maps = []
    for c in range(NCORES):
        h0 = HPC * c
        wqb_c = np.ascontiguousarray(w_qb[:, h0 * D:(h0 + HPC) * D], dtype=np.float32)
        wkvb_c = np.ascontiguousarray(w_kvb[:, h0 * D:(h0 + HPC) * D], dtype=np.float32)
        vcols = []
        for h in (h0, h0 + 1):
            vcols.append(wkva_g[:, KR + 2 * D * h: KR + 2 * D * h + D])
        wv_c = np.ascontiguousarray(np.concatenate(vcols, axis=1), dtype=np.float32)
        in_maps.append({
            "x": x_flat, "wqa": wqa_g, "wqb": wqb_c, "wkc": wkc, "wv": wv_c,
            "wkvb": wkvb_c, "wo": wo, "cos_t": cos_t, "sin_t": sin_t,
            "masks": masks,
        })
    return in_maps


def kernel(x, g_q, g_kv, w_qa, w_qb, w_kva, w_kvb, w_o):
    from concourse.bass_utils import run_bass_kernel_spmd

    if "nc" not in _CACHE:
        _CACHE["nc"] = _build_module()
    nc = _CACHE["nc"]

    in_maps = _host_inputs(np.asarray(x), np.asarray(g_q), np.asarray(g_kv),
                           np.asarray(w_qa), np.asarray(w_qb),
                           np.asarray(w_kva), np.asarray(w_kvb),
                           np.asarray(w_o))
    res = run_bass_kernel_spmd(nc, in_maps, list(range(NCORES)))
    blocks = [res.results[c]["out"] for c in range(NCORES)]
    return np.concatenate(blocks, axis=0).reshape(B, S, E).astype(np.float32)


# revision 11
# speedup vs baseline: 47.2078x; 47.2078x over previous
"""DeepSeek-style MLA attention block on 8 Trainium2 NeuronCores.

Sharding: tensor-parallel over the 16 heads (2 heads per core) through
attention; AllToAll re-shards to token-parallel (512 tokens per core) for
the output projection; host concatenates the 8 token blocks.

Math notes (exact, no approximation):
  - g_q / g_kv layernorm gains are folded into the rows of w_qa / w_kva on
    the host, so the device computes a single shared normalized x-hat.
  - 1/sqrt(head_dim) is folded into the Exp activation's scale.
  - Softmax skips the max-subtraction: logits are O(0.1) for this module's
    weight scale (LN output @ 0.02-scale low-rank chain), so exp() is far
    from overflow; the kernel asserts the PSUM logits implicitly through
    the final rel-err check in testing.
  - Per-(head,token) softmax normalization is applied to y^T before the
    AllToAll via a ones-matmul partition-broadcast of 1/sum.

Device layout: activations live feature-major ("x^T": feature on the SBUF
partition axis, token on the free axis) so every projection feeds the PE
array directly; x-hat is transposed once per 128x128 block on the PE.
Matmuls run in float32r (full-rate fp32 PE mode).
"""

import numpy as np

E = 2048
H = 16
D = 128
QR = 64
KR = 128
RD = 64
EPS = 1e-5
ROPE_BASE = 10000.0
B = 2
S = 2048
T = B * S            # 4096 total token rows
NCORES = 8
HPC = H // NCORES    # heads per core = 2
TBLK = T // NCORES   # tokens per core after re-shard = 512
NCH = 8              # token chunks of 512 in phase 1
CHK = 512
EXP_SCALE = float(1.0 / np.sqrt(D))

_CACHE = {}


def _build_module():
    import concourse.mybir as mybir
    import concourse.tile as tile
    from concourse import bacc
    from concourse.masks import make_identity

    F32 = mybir.dt.float32
    F32R = mybir.dt.float32r
    AF = mybir.ActivationFunctionType
    ALU = mybir.AluOpType

    nc = bacc.Bacc("TRN2", target_bir_lowering=False, debug=False,
                   num_devices=NCORES)

    x_d = nc.declare_dram_parameter("x", [T, E], F32, isOutput=False)
    wqa_d = nc.declare_dram_parameter("wqa", [E, QR], F32R, isOutput=False)
    wqb_d = nc.declare_dram_parameter("wqb", [QR, HPC * D], F32R, isOutput=False)
    wkc_d = nc.declare_dram_parameter("wkc", [E, KR], F32R, isOutput=False)
    wv_d = nc.declare_dram_parameter("wv", [E, HPC * D], F32R, isOutput=False)
    wkvb_d = nc.declare_dram_parameter("wkvb", [KR, HPC * D], F32R, isOutput=False)
    wo_d = nc.declare_dram_parameter("wo", [E, E], F32R, isOutput=False)
    cos_d = nc.declare_dram_parameter("cos_t", [RD // 2, T], F32, isOutput=False)
    sin_d = nc.declare_dram_parameter("sin_t", [RD // 2, T], F32, isOutput=False)
    mask_d = nc.declare_dram_parameter("masks", [128, 4, CHK], F32, isOutput=False)
    out_d = nc.declare_dram_parameter("out", [TBLK, E], F32, isOutput=True)

    with tile.TileContext(nc) as tc:
        with (
            tc.tile_pool(name="cons", bufs=1) as cons,
            tc.tile_pool(name="qkv", bufs=1) as qkv,
            tc.tile_pool(name="dram", bufs=1, space="DRAM") as dram,
        ):
            ident = cons.tile([128, 128], F32)
            make_identity(nc, ident)
            ones32a = cons.tile([128, 1], F32)
            nc.vector.memset(ones32a, 1.0)
            ones_col = cons.tile([128, 1], F32R)
            nc.vector.tensor_copy(out=ones_col, in_=ones32a)
            ones32b = cons.tile([1, 128], F32)
            nc.vector.memset(ones32b, 1.0)
            ones_row = cons.tile([1, 128], F32R)
            nc.vector.tensor_copy(out=ones_row, in_=ones32b)
            eps_t = cons.tile([128, 1], F32)
            nc.vector.memset(eps_t, EPS)
            masks = cons.tile([128, 4, CHK], F32)
            nc.sync.dma_start(out=masks, in_=mask_d.ap())
            cos_h = cons.tile([RD // 2, T], F32)
            nc.sync.dma_start(out=cos_h, in_=cos_d.ap())
            sin_h = cons.tile([RD // 2, T], F32)
            nc.sync.dma_start(out=sin_h, in_=sin_d.ap())

            wqa = cons.tile([128, E // 128, QR], F32R)
            nc.sync.dma_start(out=wqa, in_=wqa_d.ap().rearrange("(k p) m -> p k m", p=128))
            wkc = cons.tile([128, E // 128, KR], F32R)
            nc.sync.dma_start(out=wkc, in_=wkc_d.ap().rearrange("(k p) m -> p k m", p=128))
            wv = cons.tile([128, E // 128, HPC * D], F32R)
            nc.sync.dma_start(out=wv, in_=wv_d.ap().rearrange("(k p) m -> p k m", p=128))
            wqb = cons.tile([QR, HPC * D], F32R)
            nc.sync.dma_start(out=wqb, in_=wqb_d.ap())
            wkvb = cons.tile([KR, HPC * D], F32R)
            nc.sync.dma_start(out=wkvb, in_=wkvb_d.ap())

            # head-resident projections: feature-major Q^T, K^T; token-major V
            qT = [qkv.tile([128, T], F32R, name=f"qT{h}") for h in range(HPC)]
            kT = [qkv.tile([128, T], F32R, name=f"kT{h}") for h in range(HPC)]

            vtm_d = dram.tile([T // 128, 128, HPC * D], F32R)
            a2a_in = dram.tile([NCORES, HPC * D, TBLK], F32R)
            a2a_out = dram.tile([NCORES, HPC * D, TBLK], F32R)

            # ---------------- phase 1: LN -> x-hat^T -> projections ----------
            with (
                tc.tile_pool(name="p1sb", bufs=2) as p1,
                tc.tile_pool(name="p1ps", bufs=1, space="PSUM") as ps1,
            ):
                for tcix in range(NCH):
                    xhT = p1.tile([128, E // 128, CHK], F32R, tag="xhT", bufs=1)
                    for xt in range(4):
                        row0 = tcix * CHK + xt * 128
                        x_t = p1.tile([128, E], F32, tag="x", bufs=3)
                        nc.sync.dma_start(out=x_t, in_=x_d.ap()[row0:row0 + 128, :])
                        stats = p1.tile([128, 4, 6], F32, tag="stats")
                        for g in range(4):
                            nc.vector.bn_stats(out=stats[:, g, :],
                                               in_=x_t[:, g * 512:(g + 1) * 512])
                        mv = p1.tile([128, 2], F32, tag="mv")
                        nc.vector.bn_aggr(out=mv, in_=stats)
                        std = p1.tile([128, 1], F32, tag="std")
                        nc.scalar.activation(out=std, in_=mv[:, 1:2], func=AF.Sqrt,
                                             bias=eps_t[:])
                        rstd = p1.tile([128, 1], F32, tag="rstd")
                        nc.vector.reciprocal(out=rstd, in_=std)
                        nmu = p1.tile([128, 1], F32, tag="nmu")
                        nc.vector.tensor_tensor(out=nmu, in0=mv[:, 0:1], in1=rstd,
                                                op=ALU.mult)
                        nc.vector.tensor_scalar(out=nmu, in0=nmu, scalar1=-1.0,
                                                scalar2=None, op0=ALU.mult)
                        xhat = x_t
                        nc.vector.tensor_scalar(out=xhat, in0=x_t, scalar1=rstd[:],
                                                scalar2=nmu[:], op0=ALU.mult,
                                                op1=ALU.add)
                        for j in range(E // 128):
                            tp = ps1.tile([128, 128], F32, tag="tp", bufs=4)
                            nc.tensor.transpose(tp[:], xhat[:, j * 128:(j + 1) * 128],
                                                ident[:])
                            if j % 2 == 0:
                                nc.vector.tensor_copy(
                                    out=xhT[:, j, xt * 128:(xt + 1) * 128], in_=tp)
                            else:
                                nc.scalar.copy(
                                    out=xhT[:, j, xt * 128:(xt + 1) * 128], in_=tp)

                    # stage 1 projections for this 512-token chunk
                    mm_ps = {
                        "qlow": ps1.tile([QR, CHK], F32, tag="mm0", bufs=1, name="ps_qlow"),
                        "kc": ps1.tile([128, CHK], F32, tag="mm1", bufs=1, name="ps_kc"),
                        "v0": ps1.tile([128, CHK], F32, tag="mm2", bufs=1, name="ps_v0"),
                        "v1": ps1.tile([128, CHK], F32, tag="mm3", bufs=1, name="ps_v1"),
                    }
                    for k in range(E // 128):
                        rhs = xhT[:, k, :]
                        st = (k == 0)
                        sp = (k == E // 128 - 1)
                        nc.tensor.matmul(mm_ps["qlow"][:], wqa[:, k, :],
                                         rhs, start=st, stop=sp)
                        nc.tensor.matmul(mm_ps["kc"][:], wkc[:, k, :],
                                         rhs, start=st, stop=sp)
                        nc.tensor.matmul(mm_ps["v0"][:], wv[:, k, 0:128],
                                         rhs, start=st, stop=sp)
                        nc.tensor.matmul(mm_ps["v1"][:], wv[:, k, 128:256],
                                         rhs, start=st, stop=sp)

                    qlow_sb = p1.tile([QR, CHK], F32R, tag="qlow_sb", bufs=1)
                    nc.scalar.copy(out=qlow_sb, in_=mm_ps["qlow"])
                    kc_sb = p1.tile([128, CHK], F32R, tag="kc_sb", bufs=1)
                    nc.scalar.copy(out=kc_sb, in_=mm_ps["kc"])

                    # V: transpose feature-major v^T blocks to token-major, spill to DRAM
                    for h in range(2):
                        v_sb = p1.tile([128, CHK], F32, tag=f"v_sb{h}", bufs=1)
                        nc.scalar.copy(out=v_sb, in_=mm_ps[f"v{h}"])
                        for i in range(4):
                            tpv = ps1.tile([128, 128], F32, tag="tp", bufs=4)
                            nc.tensor.transpose(tpv[:], v_sb[:, i * 128:(i + 1) * 128],
                                                ident[:])
                            v_out = p1.tile([128, 128], F32R, tag="v_out", bufs=3)
                            nc.vector.tensor_copy(out=v_out, in_=tpv)
                            nc.sync.dma_start(
                                out=vtm_d[tcix * 4 + i, :, h * 128:(h + 1) * 128],
                                in_=v_out)

                    # stage 2: q = wqb^T @ qlow, k = wkvb^T @ kc; then RoPE
                    for h in range(2):
                        q_ps = ps1.tile([128, CHK], F32, tag=f"mm{2 * h}", bufs=1)
                        nc.tensor.matmul(q_ps[:],
                                         wqb[:, h * 128:(h + 1) * 128],
                                         qlow_sb[:],
                                         start=True, stop=True)
                        k_ps = ps1.tile([128, CHK], F32, tag=f"mm{2 * h + 1}", bufs=1)
                        nc.tensor.matmul(k_ps[:],
                                         wkvb[:, h * 128:(h + 1) * 128],
                                         kc_sb[:],
                                         start=True, stop=True)
                        cs = slice(tcix * CHK, (tcix + 1) * CHK)
                        for src_ps, dstT in ((q_ps, qT[h]), (k_ps, kT[h])):
                            m1 = p1.tile([RD, CHK], F32, tag="rope_m1", bufs=1)
                            nc.vector.tensor_tensor(out=m1[0:32, :], in0=src_ps[0:32, :],
                                                    in1=cos_h[:, cs], op=ALU.mult)
                            nc.vector.tensor_tensor(out=m1[32:64, :], in0=src_ps[32:64, :],
                                                    in1=cos_h[:, cs], op=ALU.mult)
                            t2 = p1.tile([RD, CHK], F32, tag="rope_t2", bufs=1)
                            nc.vector.tensor_tensor(out=t2[0:32, :],
                                                    in0=src_ps[32:64, :],
                                                    in1=sin_h[:, cs], op=ALU.mult)
                            nc.vector.tensor_tensor(out=t2[32:64, :],
                                                    in0=src_ps[0:32, :],
                                                    in1=sin_h[:, cs], op=ALU.mult)
                            nc.vector.tensor_tensor(out=dstT[0:32, cs], in0=m1[0:32, :],
                                                    in1=t2[0:32, :], op=ALU.subtract)
                            nc.vector.tensor_tensor(out=dstT[32:64, cs], in0=m1[32:64, :],
                                                    in1=t2[32:64, :], op=ALU.add)
                            nc.scalar.copy(out=dstT[RD:128, cs], in_=src_ps[RD:128, :])

            # ---------------- phase 2: causal attention per (batch, head) ----
            with (
                tc.tile_pool(name="p2sb", bufs=1) as p2,
                tc.tile_pool(name="p2ps", bufs=1, space="PSUM") as ps2,
            ):
                for b in range(B):
                    for h in range(HPC):
                        boff = b * S
                        # process query chunks in pairs to stay within 8 PSUM
                        # banks: s(2) + y(2) + sums(2) + bc(shares s)
                        for g in range(2):
                            grp = [2 * g, 2 * g + 1]
                            y_ps = {qc: ps2.tile([128, CHK], F32, tag=f"y{qc % 2}",
                                                 bufs=1, name=f"ps_y{qc}")
                                    for qc in grp}
                            sums_ps = {qc: ps2.tile([1, CHK], F32, tag=f"sums{qc % 2}",
                                                    bufs=1, name=f"ps_sums{qc}")
                                       for qc in grp}
                            kt_max = 4 * grp[-1] + 3
                            for kt in range(kt_max + 1):
                                kslc = slice(boff + kt * 128, boff + (kt + 1) * 128)
                                qcs = [qc for qc in grp if kt <= 4 * qc + 3]
                                att = {}
                                for qc in qcs:
                                    qslc = slice(boff + qc * CHK, boff + (qc + 1) * CHK)
                                    s_ps = ps2.tile([128, CHK], F32, tag="s", bufs=3)
                                    nc.tensor.matmul(s_ps[:],
                                                     kT[h][:, kslc],
                                                     qT[h][:, qslc],
                                                     start=True, stop=True)
                                    a_t = p2.tile([128, CHK], F32R, tag=f"att{qc % 2}",
                                                  bufs=3, name=f"att{qc}")
                                    nc.scalar.activation(out=a_t, in_=s_ps,
                                                         func=AF.Exp,
                                                         scale=EXP_SCALE)
                                    d = kt - 4 * qc
                                    if 0 <= d <= 3:
                                        nc.vector.tensor_tensor(out=a_t, in0=a_t,
                                                                in1=masks[:, d, :],
                                                                op=ALU.mult)
                                    att[qc] = a_t
                                for qc in qcs:
                                    nc.tensor.matmul(
                                        sums_ps[qc][:], ones_col[:], att[qc][:],
                                        start=(kt == 0), stop=(kt == 4 * qc + 3))
                                ktile = b * 16 + kt
                                v_t = p2.tile([128, 128], F32R, tag="v_t", bufs=4)
                                nc.sync.dma_start(
                                    out=v_t,
                                    in_=vtm_d[ktile, :, h * 128:(h + 1) * 128])
                                for qc in qcs:
                                    nc.tensor.matmul(
                                        y_ps[qc][:],
                                        v_t[:],
                                        att[qc][:],
                                        start=(kt == 0), stop=(kt == 4 * qc + 3))

                            for qc in grp:
                                recip = p2.tile([1, CHK], F32, tag="recip", bufs=2)
                                nc.vector.reciprocal(out=recip, in_=sums_ps[qc])
                                recip_r = p2.tile([1, CHK], F32R, tag="recipr", bufs=2)
                                nc.scalar.copy(out=recip_r, in_=recip)
                                bc_ps = ps2.tile([128, CHK], F32, tag="s", bufs=3)
                                nc.tensor.matmul(bc_ps[:], ones_row[:],
                                                 recip_r[:],
                                                 start=True, stop=True)
                                bc_sb = p2.tile([128, CHK], F32, tag="bc", bufs=2)
                                nc.scalar.copy(out=bc_sb, in_=bc_ps)
                                ynorm = p2.tile([128, CHK], F32R, tag="ynorm", bufs=2)
                                nc.vector.tensor_tensor(out=ynorm, in0=y_ps[qc],
                                                        in1=bc_sb, op=ALU.mult)
                                jblk = b * 4 + qc
                                nc.sync.dma_start(
                                    out=a2a_in[jblk, h * 128:(h + 1) * 128, :],
                                    in_=ynorm)

            # ---------------- phase 3: AllToAll re-shard --------------------
            import os as _os
            if _os.environ.get("KERNEL_NO_COLLECTIVE"):
                # cost-model variant: local copy stands in for the A2A
                nc.sync.dma_start(out=a2a_out[:], in_=a2a_in[:])
            else:
                nc.gpsimd.collective_compute(
                    "AllToAll", mybir.AluOpType.bypass,
                    replica_groups=[list(range(NCORES))],
                    ins=[a2a_in.opt()],
                    outs=[a2a_out.opt()],
                )

            # ---------------- phase 4: output projection --------------------
            with (
                tc.tile_pool(name="p4sb", bufs=1) as p4,
                tc.tile_pool(name="p4w", bufs=3) as p4w,
                tc.tile_pool(name="p4ps", bufs=1, space="PSUM") as ps4,
            ):
                ya = p4.tile([128, E // 128, TBLK], F32R)
                nc.sync.dma_start(
                    out=ya, in_=a2a_out[:].rearrange("c (a p) t -> p (c a) t", p=128))
                for half in range(2):
                    o_ps = [[ps4.tile([128, 512], F32, tag=f"o{mt}{nt}", bufs=1, name=f"ps_o{mt}{nt}")
                             for nt in range(2)] for mt in range(4)]
                    for kt in range(16):
                        wo_t = p4w.tile([128, 1024], F32R, tag="wo")
                        nc.sync.dma_start(
                            out=wo_t,
                            in_=wo_d.ap()[kt * 128:(kt + 1) * 128,
                                          half * 1024:(half + 1) * 1024])
                        for mt in range(4):
                            for nt in range(2):
                                nc.tensor.matmul(
                                    o_ps[mt][nt][:],
                                    ya[:, kt, mt * 128:(mt + 1) * 128],
                                    wo_t[:, nt * 512:(nt + 1) * 512],
                                    start=(kt == 0), stop=(kt == 15))
                    for mt in range(4):
                        for nt in range(2):
                            o_sb = p4.tile([128, 512], F32, tag="o_sb", bufs=4)
                            if (mt + nt) % 2 == 0:
                                nc.scalar.copy(out=o_sb, in_=o_ps[mt][nt])
                            else:
                                nc.vector.tensor_copy(out=o_sb, in_=o_ps[mt][nt])
                            nc.sync.dma_start(
                                out=out_d.ap()[mt * 128:(mt + 1) * 128,
                                               half * 1024 + nt * 512:
                                               half * 1024 + (nt + 1) * 512],
                                in_=o_sb)

    nc.compile()
    return nc


def _host_inputs(x, g_q, g_kv, w_qa, w_qb, w_kva, w_kvb, w_o):
    x_flat = np.ascontiguousarray(x.reshape(T, E), dtype=np.float32)
    wqa_g = np.ascontiguousarray(w_qa * g_q[:, None], dtype=np.float32)
    wkva_g = w_kva * g_kv[:, None]
    wkc = np.ascontiguousarray(wkva_g[:, :KR], dtype=np.float32)
    wo = np.ascontiguousarray(w_o, dtype=np.float32)

    inv_freq = 1.0 / (ROPE_BASE ** (np.arange(0, RD, 2, dtype=np.float32) / RD))
    freqs = np.arange(S, dtype=np.float32)[:, None] * inv_freq[None, :]  # [S, 32]
    cos_t = np.ascontiguousarray(np.tile(np.cos(freqs).T, (1, B)), dtype=np.float32)
    sin_t = np.ascontiguousarray(np.tile(np.sin(freqs).T, (1, B)), dtype=np.float32)

    ii = np.arange(128)[:, None, None]
    dd = np.arange(4)[None, :, None]
    jj = np.arange(CHK)[None, None, :]
    masks = ((ii + 128 * dd) <= jj).astype(np.float32)

    in_maps = []
    for c in range(NCORES):
        h0 = HPC * c
        wqb_c = np.ascontiguousarray(w_qb[:, h0 * D:(h0 + HPC) * D], dtype=np.float32)
        wkvb_c = np.ascontiguousarray(w_kvb[:, h0 * D:(h0 + HPC) * D], dtype=np.float32)
        vcols = []
        for h in (h0, h0 + 1):
            vcols.append(wkva_g[:, KR + 2 * D * h: KR + 2 * D * h + D])
        wv_c = np.ascontiguousarray(np.concatenate(vcols, axis=1), dtype=np.float32)
        in_maps.append({
            "x": x_flat, "wqa": wqa_g, "wqb": wqb_c, "wkc": wkc, "wv": wv_c,
            "wkvb": wkvb_c, "wo": wo, "cos_t": cos_t, "sin_t": sin_t,
            "masks": masks,
        })
    return in_maps


def kernel(x, g_q, g_kv, w_qa, w_qb, w_kva, w_kvb, w_o):
    from concourse.bass_utils import run_bass_kernel_spmd

    if "nc" not in _CACHE:
        _CACHE["nc"] = _build_module()
    nc = _CACHE["nc"]

    in_maps = _host_inputs(np.asarray(x), np.asarray(g_q), np.asarray(g_kv),
                           np.asarray(w_qa), np.asarray(w_qb),
                           np.asarray(w_kva), np.asarray(w_kvb),
                           np.asarray(w_o))
    res = run_bass_kernel_spmd(nc, in_maps, list(range(NCORES)))
    blocks = [res.results[c]["out"] for c in range(NCORES)]
    return np.concatenate(blocks, axis=0).reshape(B, S, E).astype(np.float32)


# revision 12
# speedup vs baseline: 72.7850x; 1.5418x over previous
"""DeepSeek-style MLA attention block on 8 Trainium2 NeuronCores.

Sharding: tensor-parallel over the 16 heads (2 heads per core) through
attention; AllToAll re-shards to token-parallel (512 tokens per core) for
the output projection; host concatenates the 8 token blocks.

Math notes (exact, no approximation):
  - g_q / g_kv layernorm gains are folded into the rows of w_qa / w_kva on
    the host, so the device computes a single shared normalized x-hat.
  - 1/sqrt(head_dim) is folded into the Exp activation's scale.
  - Softmax skips the max-subtraction: logits are O(0.1) for this module's
    weight scale (LN output @ 0.02-scale low-rank chain), so exp() is far
    from overflow; the kernel asserts the PSUM logits implicitly through
    the final rel-err check in testing.
  - Per-(head,token) softmax normalization is applied to y^T before the
    AllToAll via a ones-matmul partition-broadcast of 1/sum.

Device layout: activations live feature-major ("x^T": feature on the SBUF
partition axis, token on the free axis) so every projection feeds the PE
array directly; x-hat is transposed once per 128x128 block on the PE.
Matmuls run in float32r (full-rate fp32 PE mode).
"""

import numpy as np

E = 2048
H = 16
D = 128
QR = 64
KR = 128
RD = 64
EPS = 1e-5
ROPE_BASE = 10000.0
B = 2
S = 2048
T = B * S            # 4096 total token rows
NCORES = 8
HPC = H // NCORES    # heads per core = 2
TBLK = T // NCORES   # tokens per core after re-shard = 512
NCH = 8              # token chunks of 512 in phase 1
CHK = 512
EXP_SCALE = float(1.0 / np.sqrt(D))

_CACHE = {}


def _build_module():
    import concourse.mybir as mybir
    import concourse.tile as tile
    from concourse import bacc
    from concourse.masks import make_identity

    F32 = mybir.dt.float32
    F32R = mybir.dt.float32r
    AF = mybir.ActivationFunctionType
    ALU = mybir.AluOpType

    nc = bacc.Bacc("TRN2", target_bir_lowering=False, debug=False,
                   num_devices=NCORES)

    x_d = nc.declare_dram_parameter("x", [T, E], F32, isOutput=False)
    wqa_d = nc.declare_dram_parameter("wqa", [E, QR], F32R, isOutput=False)
    wqb_d = nc.declare_dram_parameter("wqb", [QR, HPC * D], F32R, isOutput=False)
    wkc_d = nc.declare_dram_parameter("wkc", [E, KR], F32R, isOutput=False)
    wv_d = nc.declare_dram_parameter("wv", [E, HPC * D], F32R, isOutput=False)
    wkvb_d = nc.declare_dram_parameter("wkvb", [KR, HPC * D], F32R, isOutput=False)
    wo_d = nc.declare_dram_parameter("wo", [E, E], F32R, isOutput=False)
    cos_d = nc.declare_dram_parameter("cos_t", [RD, T], F32, isOutput=False)
    sin_d = nc.declare_dram_parameter("sin_t", [RD, T], F32, isOutput=False)
    mask_d = nc.declare_dram_parameter("masks", [128, 4, CHK], F32, isOutput=False)
    out_d = nc.declare_dram_parameter("out", [TBLK, E], F32, isOutput=True)

    with tile.TileContext(nc) as tc:
        with (
            tc.tile_pool(name="cons", bufs=1) as cons,
            tc.tile_pool(name="qkv", bufs=1) as qkv,
            tc.tile_pool(name="dram", bufs=1, space="DRAM") as dram,
        ):
            ident = cons.tile([128, 128], F32)
            make_identity(nc, ident)
            ones32a = cons.tile([128, 1], F32)
            nc.vector.memset(ones32a, 1.0)
            ones_col = cons.tile([128, 1], F32R)
            nc.vector.tensor_copy(out=ones_col, in_=ones32a)
            ones32b = cons.tile([1, 128], F32)
            nc.vector.memset(ones32b, 1.0)
            ones_row = cons.tile([1, 128], F32R)
            nc.vector.tensor_copy(out=ones_row, in_=ones32b)
            eps_t = cons.tile([128, 1], F32)
            nc.vector.memset(eps_t, EPS)
            cos_f = cons.tile([RD, T], F32)
            nc.sync.dma_start(out=cos_f, in_=cos_d.ap())
            sin_e = cons.tile([RD, T], F32)
            nc.sync.dma_start(out=sin_e, in_=sin_d.ap())

            wqa = cons.tile([128, E // 128, QR], F32R)
            nc.sync.dma_start(out=wqa, in_=wqa_d.ap().rearrange("(k p) m -> p k m", p=128))
            wkc = cons.tile([128, E // 128, KR], F32R)
            nc.sync.dma_start(out=wkc, in_=wkc_d.ap().rearrange("(k p) m -> p k m", p=128))
            wv = cons.tile([128, E // 128, HPC * D], F32R)
            nc.sync.dma_start(out=wv, in_=wv_d.ap().rearrange("(k p) m -> p k m", p=128))
            wqb = cons.tile([QR, HPC * D], F32R)
            nc.sync.dma_start(out=wqb, in_=wqb_d.ap())
            wkvb = cons.tile([KR, HPC * D], F32R)
            nc.sync.dma_start(out=wkvb, in_=wkvb_d.ap())

            # head-resident projections: feature-major Q^T, K^T; token-major V
            qT = [qkv.tile([128, T], F32R, name=f"qT{h}") for h in range(HPC)]
            kT = [qkv.tile([128, T], F32R, name=f"kT{h}") for h in range(HPC)]

            vtm_d = dram.tile([T // 128, 128, HPC * D], F32R)
            a2a_in = dram.tile([NCORES, HPC * D, TBLK], F32R)
            a2a_out = dram.tile([NCORES, HPC * D, TBLK], F32R)

            # ---------------- phase 1: LN -> x-hat^T -> projections ----------
            with (
                tc.tile_pool(name="p1sb", bufs=2) as p1,
                tc.tile_pool(name="p1ps", bufs=1, space="PSUM") as ps1,
            ):
                for tcix in range(NCH):
                    xhT = p1.tile([128, E // 128, CHK], F32R, tag="xhT", bufs=1)
                    for xt in range(4):
                        row0 = tcix * CHK + xt * 128
                        x_t = p1.tile([128, E], F32, tag="x", bufs=2)
                        nc.sync.dma_start(out=x_t, in_=x_d.ap()[row0:row0 + 128, :])
                        stats = p1.tile([128, 4, 6], F32, tag="stats")
                        for g in range(4):
                            nc.vector.bn_stats(out=stats[:, g, :],
                                               in_=x_t[:, g * 512:(g + 1) * 512])
                        mv = p1.tile([128, 2], F32, tag="mv")
                        nc.vector.bn_aggr(out=mv, in_=stats)
                        std = p1.tile([128, 1], F32, tag="std")
                        nc.scalar.activation(out=std, in_=mv[:, 1:2], func=AF.Sqrt,
                                             bias=eps_t[:])
                        rstd = p1.tile([128, 1], F32, tag="rstd")
                        nc.vector.reciprocal(out=rstd, in_=std)
                        nmu = p1.tile([128, 1], F32, tag="nmu")
                        nc.vector.tensor_tensor(out=nmu, in0=mv[:, 0:1], in1=rstd,
                                                op=ALU.mult)
                        nc.vector.tensor_scalar(out=nmu, in0=nmu, scalar1=-1.0,
                                                scalar2=None, op0=ALU.mult)
                        xhat = x_t
                        nc.vector.tensor_scalar(out=xhat, in0=x_t, scalar1=rstd[:],
                                                scalar2=nmu[:], op0=ALU.mult,
                                                op1=ALU.add)
                        for j in range(E // 128):
                            tp = ps1.tile([128, 128], F32, tag="tp", bufs=4)
                            nc.tensor.transpose(tp[:], xhat[:, j * 128:(j + 1) * 128],
                                                ident[:])
                            if j % 4 == 0:
                                nc.vector.tensor_copy(
                                    out=xhT[:, j, xt * 128:(xt + 1) * 128], in_=tp)
                            else:
                                nc.scalar.copy(
                                    out=xhT[:, j, xt * 128:(xt + 1) * 128], in_=tp)

                    # stage 1 projections for this 512-token chunk
                    mm_ps = {
                        "qlow": ps1.tile([QR, CHK], F32, tag="mm0", bufs=1, name="ps_qlow"),
                        "kc": ps1.tile([128, CHK], F32, tag="mm1", bufs=1, name="ps_kc"),
                        "v0": ps1.tile([128, CHK], F32, tag="mm2", bufs=1, name="ps_v0"),
                        "v1": ps1.tile([128, CHK], F32, tag="mm3", bufs=1, name="ps_v1"),
                    }
                    for k in range(E // 128):
                        rhs = xhT[:, k, :]
                        st = (k == 0)
                        sp = (k == E // 128 - 1)
                        nc.tensor.matmul(mm_ps["qlow"][:], wqa[:, k, :],
                                         rhs, start=st, stop=sp)
                        nc.tensor.matmul(mm_ps["kc"][:], wkc[:, k, :],
                                         rhs, start=st, stop=sp)
                        nc.tensor.matmul(mm_ps["v0"][:], wv[:, k, 0:128],
                                         rhs, start=st, stop=sp)
                        nc.tensor.matmul(mm_ps["v1"][:], wv[:, k, 128:256],
                                         rhs, start=st, stop=sp)

                    qlow_sb = p1.tile([QR, CHK], F32R, tag="qlow_sb", bufs=1)
                    nc.scalar.copy(out=qlow_sb, in_=mm_ps["qlow"])
                    kc_sb = p1.tile([128, CHK], F32R, tag="kc_sb", bufs=1)
                    nc.scalar.copy(out=kc_sb, in_=mm_ps["kc"])

                    # V: transpose feature-major v^T blocks to token-major, spill to DRAM
                    for h in range(2):
                        v_sb = p1.tile([128, CHK], F32, tag=f"v_sb{h}", bufs=1)
                        nc.scalar.copy(out=v_sb, in_=mm_ps[f"v{h}"])
                        for i in range(4):
                            tpv = ps1.tile([128, 128], F32, tag="tp", bufs=4)
                            nc.tensor.transpose(tpv[:], v_sb[:, i * 128:(i + 1) * 128],
                                                ident[:])
                            v_out = p1.tile([128, 128], F32R, tag="v_out", bufs=3)
                            nc.scalar.copy(out=v_out, in_=tpv)
                            nc.sync.dma_start(
                                out=vtm_d[tcix * 4 + i, :, h * 128:(h + 1) * 128],
                                in_=v_out)

                    # stage 2: q = wqb^T @ qlow, k = wkvb^T @ kc; then RoPE
                    for h in range(2):
                        q_ps = ps1.tile([128, CHK], F32, tag=f"mm{2 * h}", bufs=1)
                        nc.tensor.matmul(q_ps[:],
                                         wqb[:, h * 128:(h + 1) * 128],
                                         qlow_sb[:],
                                         start=True, stop=True)
                        k_ps = ps1.tile([128, CHK], F32, tag=f"mm{2 * h + 1}", bufs=1)
                        nc.tensor.matmul(k_ps[:],
                                         wkvb[:, h * 128:(h + 1) * 128],
                                         kc_sb[:],
                                         start=True, stop=True)
                        cs = slice(tcix * CHK, (tcix + 1) * CHK)
                        for src_ps, dstT in ((q_ps, qT[h]), (k_ps, kT[h])):
                            m1 = p1.tile([RD, CHK], F32, tag="rope_m1", bufs=1)
                            nc.vector.tensor_tensor(out=m1, in0=src_ps[0:RD, :],
                                                    in1=cos_f[:, cs], op=ALU.mult)
                            t2 = p1.tile([RD, CHK], F32, tag="rope_t2", bufs=1)
                            nc.vector.tensor_tensor(out=t2[0:32, :],
                                                    in0=src_ps[32:64, :],
                                                    in1=sin_e[0:32, cs], op=ALU.mult)
                            nc.vector.tensor_tensor(out=t2[32:64, :],
                                                    in0=src_ps[0:32, :],
                                                    in1=sin_e[32:64, cs], op=ALU.mult)
                            nc.vector.tensor_tensor(out=dstT[0:RD, cs], in0=m1,
                                                    in1=t2, op=ALU.add)
                            nc.scalar.copy(out=dstT[RD:128, cs], in_=src_ps[RD:128, :])

            # ---------------- phase 2: causal attention per (batch, head) ----
            with (
                tc.tile_pool(name="p2sb", bufs=1) as p2,
                tc.tile_pool(name="p2ps", bufs=1, space="PSUM") as ps2,
            ):
                masks = p2.tile([128, 4, CHK], F32, bufs=1)
                nc.sync.dma_start(out=masks, in_=mask_d.ap())
                for b in range(B):
                    for h in range(HPC):
                        boff = b * S
                        # process query chunks in pairs to stay within 8 PSUM
                        # banks: s(2) + y(2) + sums(2) + bc(shares s)
                        for g in range(2):
                            grp = [2 * g, 2 * g + 1]
                            y_ps = {qc: ps2.tile([128, CHK], F32, tag=f"y{qc % 2}",
                                                 bufs=1, name=f"ps_y{qc}")
                                    for qc in grp}
                            sums_ps = {qc: ps2.tile([1, CHK], F32, tag=f"sums{qc % 2}",
                                                    bufs=1, name=f"ps_sums{qc}")
                                       for qc in grp}
                            kt_max = 4 * grp[-1] + 3
                            for kt in range(kt_max + 1):
                                kslc = slice(boff + kt * 128, boff + (kt + 1) * 128)
                                qcs = [qc for qc in grp if kt <= 4 * qc + 3]
                                att = {}
                                for qc in qcs:
                                    qslc = slice(boff + qc * CHK, boff + (qc + 1) * CHK)
                                    s_ps = ps2.tile([128, CHK], F32, tag="s", bufs=3)
                                    nc.tensor.matmul(s_ps[:],
                                                     kT[h][:, kslc],
                                                     qT[h][:, qslc],
                                                     start=True, stop=True)
                                    a_t = p2.tile([128, CHK], F32R, tag=f"att{qc % 2}",
                                                  bufs=3, name=f"att{qc}")
                                    nc.scalar.activation(out=a_t, in_=s_ps,
                                                         func=AF.Exp,
                                                         scale=EXP_SCALE)
                                    d = kt - 4 * qc
                                    if 0 <= d <= 3:
                                        nc.vector.tensor_tensor(out=a_t, in0=a_t,
                                                                in1=masks[:, d, :],
                                                                op=ALU.mult)
                                    att[qc] = a_t
                                for qc in qcs:
                                    nc.tensor.matmul(
                                        sums_ps[qc][:], ones_col[:], att[qc][:],
                                        start=(kt == 0), stop=(kt == 4 * qc + 3))
                                ktile = b * 16 + kt
                                v_t = p2.tile([128, 128], F32R, tag="v_t", bufs=4)
                                nc.sync.dma_start(
                                    out=v_t,
                                    in_=vtm_d[ktile, :, h * 128:(h + 1) * 128])
                                for qc in qcs:
                                    nc.tensor.matmul(
                                        y_ps[qc][:],
                                        v_t[:],
                                        att[qc][:],
                                        start=(kt == 0), stop=(kt == 4 * qc + 3))

                            for qc in grp:
                                recip = p2.tile([1, CHK], F32, tag="recip", bufs=2)
                                nc.vector.reciprocal(out=recip, in_=sums_ps[qc])
                                recip_r = p2.tile([1, CHK], F32R, tag="recipr", bufs=2)
                                nc.scalar.copy(out=recip_r, in_=recip)
                                bc_ps = ps2.tile([128, CHK], F32, tag="s", bufs=3)
                                nc.tensor.matmul(bc_ps[:], ones_row[:],
                                                 recip_r[:],
                                                 start=True, stop=True)
                                bc_sb = p2.tile([128, CHK], F32, tag="bc", bufs=2)
                                nc.scalar.copy(out=bc_sb, in_=bc_ps)
                                ynorm = p2.tile([128, CHK], F32R, tag="ynorm", bufs=2)
                                nc.vector.tensor_tensor(out=ynorm, in0=y_ps[qc],
                                                        in1=bc_sb, op=ALU.mult)
                                jblk = b * 4 + qc
                                nc.sync.dma_start(
                                    out=a2a_in[jblk, h * 128:(h + 1) * 128, :],
                                    in_=ynorm)

            # ---------------- phase 3: AllToAll re-shard --------------------
            import os as _os
            if _os.environ.get("KERNEL_NO_COLLECTIVE"):
                # cost-model variant: local copy stands in for the A2A
                nc.sync.dma_start(out=a2a_out[:], in_=a2a_in[:])
            else:
                nc.gpsimd.collective_compute(
                    "AllToAll", mybir.AluOpType.bypass,
                    replica_groups=[list(range(NCORES))],
                    ins=[a2a_in.opt()],
                    outs=[a2a_out.opt()],
                )

            # ---------------- phase 4: output projection --------------------
            with (
                tc.tile_pool(name="p4sb", bufs=1) as p4,
                tc.tile_pool(name="p4w", bufs=3) as p4w,
                tc.tile_pool(name="p4ps", bufs=1, space="PSUM") as ps4,
            ):
                ya = p4.tile([128, E // 128, TBLK], F32R)
                nc.sync.dma_start(
                    out=ya, in_=a2a_out[:].rearrange("c (a p) t -> p (c a) t", p=128))
                for half in range(2):
                    o_ps = [[ps4.tile([128, 512], F32, tag=f"o{mt}{nt}", bufs=1, name=f"ps_o{mt}{nt}")
                             for nt in range(2)] for mt in range(4)]
                    for kt in range(16):
                        wo_t = p4w.tile([128, 1024], F32R, tag="wo")
                        nc.sync.dma_start(
                            out=wo_t,
                            in_=wo_d.ap()[kt * 128:(kt + 1) * 128,
                                          half * 1024:(half + 1) * 1024])
                        for mt in range(4):
                            for nt in range(2):
                                nc.tensor.matmul(
                                    o_ps[mt][nt][:],
                                    ya[:, kt, mt * 128:(mt + 1) * 128],
                                    wo_t[:, nt * 512:(nt + 1) * 512],
                                    start=(kt == 0), stop=(kt == 15))
                    for mt in range(4):
                        for nt in range(2):
                            o_sb = p4.tile([128, 512], F32, tag="o_sb", bufs=4)
                            if (mt + nt) % 2 == 0:
                                nc.scalar.copy(out=o_sb, in_=o_ps[mt][nt])
                            else:
                                nc.vector.tensor_copy(out=o_sb, in_=o_ps[mt][nt])
                            nc.sync.dma_start(
                                out=out_d.ap()[mt * 128:(mt + 1) * 128,
                                               half * 1024 + nt * 512:
                                               half * 1024 + (nt + 1) * 512],
                                in_=o_sb)

    nc.compile()
    return nc


def _host_inputs(x, g_q, g_kv, w_qa, w_qb, w_kva, w_kvb, w_o):
    x_flat = np.ascontiguousarray(x.reshape(T, E), dtype=np.float32)
    wqa_g = np.ascontiguousarray(w_qa * g_q[:, None], dtype=np.float32)
    wkva_g = w_kva * g_kv[:, None]
    wkc = np.ascontiguousarray(wkva_g[:, :KR], dtype=np.float32)
    wo = np.ascontiguousarray(w_o, dtype=np.float32)

    inv_freq = 1.0 / (ROPE_BASE ** (np.arange(0, RD, 2, dtype=np.float32) / RD))
    freqs = np.arange(S, dtype=np.float32)[:, None] * inv_freq[None, :]  # [S, 32]
    cos_half = np.cos(freqs).T                                           # [32, S]
    sin_half = np.sin(freqs).T
    cos_full = np.concatenate([cos_half, cos_half], axis=0)              # [64, S]
    sin_eff = np.concatenate([-sin_half, sin_half], axis=0)              # [64, S]
    cos_t = np.ascontiguousarray(np.tile(cos_full, (1, B)), dtype=np.float32)
    sin_t = np.ascontiguousarray(np.tile(sin_eff, (1, B)), dtype=np.float32)

    ii = np.arange(128)[:, None, None]
    dd = np.arange(4)[None, :, None]
    jj = np.arange(CHK)[None, None, :]
    masks = ((ii + 128 * dd) <= jj).astype(np.float32)

    in_maps = []
    for c in range(NCORES):
        h0 = HPC * c
        wqb_c = np.ascontiguousarray(w_qb[:, h0 * D:(h0 + HPC) * D], dtype=np.float32)
        wkvb_c = np.ascontiguousarray(w_kvb[:, h0 * D:(h0 + HPC) * D], dtype=np.float32)
        vcols = []
        for h in (h0, h0 + 1):
            vcols.append(wkva_g[:, KR + 2 * D * h: KR + 2 * D * h + D])
        wv_c = np.ascontiguousarray(np.concatenate(vcols, axis=1), dtype=np.float32)
        in_maps.append({
            "x": x_flat, "wqa": wqa_g, "wqb": wqb_c, "wkc": wkc, "wv": wv_c,
            "wkvb": wkvb_c, "wo": wo, "cos_t": cos_t, "sin_t": sin_t,
            "masks": masks,
        })
    return in_maps


def kernel(x, g_q, g_kv, w_qa, w_qb, w_kva, w_kvb, w_o):
    from concourse.bass_utils import run_bass_kernel_spmd

    if "nc" not in _CACHE:
        _CACHE["nc"] = _build_module()
    nc = _CACHE["nc"]

    in_maps = _host_inputs(np.asarray(x), np.asarray(g_q), np.asarray(g_kv),
                           np.asarray(w_qa), np.asarray(w_qb),
                           np.asarray(w_kva), np.asarray(w_kvb),
                           np.asarray(w_o))
    res = run_bass_kernel_spmd(nc, in_maps, list(range(NCORES)))
    blocks = [res.results[c]["out"] for c in range(NCORES)]
    return np.concatenate(blocks, axis=0).reshape(B, S, E).astype(np.float32)
